# revision 32
# baseline (speedup 1.0000x reference)
"""Multi-head self-attention (B=4, S=2048, hidden=1024, 16 heads, d_k=64,
causal) on 8 Trainium2 NeuronCores.

Sharding: core c handles batch b = c//2 and head-group hg = c%2 (8 heads =
512 hidden dims). Each core computes Q/K/V for its heads, causal attention,
and a partial output projection against its wo column-slice; the host sums
the two partials per batch and adds bo.

v5: all-bf16, fully software-pipelined. Per 512-token chunk c the kernel
runs attention(qc=c) with the QKV projections of chunk c+1 and the output
projection of chunk c-1 emitted as paced filler between attention score
groups. The attention stream alone is ACT(exp)-bound at the group level;
without independent filler matmuls the PE idles in sub-us gaps, de-ramps
to the mid p-state, and every matmul doubles.

Device layouts (SBUF is [128 partitions, free]):
  x^T   [in=8*128, tok]      host-transposed activations
  Q^T/K^T [dout, tok-chunk]  head h occupies rows (h%2)*64..+64 of dblk h//2
  V     [tok, head, d_k+1]   65th column is ones so the PV matmul also
                             produces the softmax denominator row
  scores are computed transposed: S^T[k, q] = K @ Q^T, so softmax's sum
  over keys becomes a matmul contraction instead of a partition reduction.
  Causal masking is a post-exp multiply by a 0/1 triangle (bf16, SBUF) on
  the diagonal 128x128 blocks. Odd heads write attention rows 64..127
  directly: 64-partition DVE ops run on banks 0-1, whose output crossbar
  reaches either SBUF half while reads follow the source access pattern.
"""

import os
import sys

for _p in (
    "/root/.axon_site",
    "/root/.axon_site/_ro/trn_rl_repo",
    "/root/.axon_site/_ro/pypackages",
    "/opt/trn_rl_repo",
):
    if os.path.isdir(_p) and _p not in sys.path:
        sys.path.append(_p)

import ml_dtypes
import numpy as np

import concourse.mybir as mybir
import concourse.tile as tile
from concourse import bacc
from concourse.bass import ts
from concourse.bass_utils import run_bass_kernel_spmd

F32 = mybir.dt.float32
BF16 = mybir.dt.bfloat16
AF = mybir.ActivationFunctionType
ALU = mybir.AluOpType

B, S, HID = 4, 2048, 1024
HEADS, DK = 16, 64
NCORES = 8
HPC = HEADS // 2          # 8 heads per core
HSL = HPC * DK            # 512-dim hidden slice per core
TC = 512                  # token/query chunk
NTC = S // TC             # 4
GRP = 2                   # k-blocks per PSUM score group


def build_nc():
    nc = bacc.Bacc("TRN2", target_bir_lowering=False, debug=False)

    # Inputs are host-packed so each partition's DMA data is one
    # contiguous run (128 descriptors/tensor instead of 1024): element
    # [p, ...] of the device view is DRAM row p.
    xT = nc.dram_tensor("xT", [128, NTC * 8 * TC], BF16, kind="ExternalInput").ap()
    wqT = nc.dram_tensor("wqT", [128, 8 * HSL], BF16, kind="ExternalInput").ap()
    wkT = nc.dram_tensor("wkT", [128, 8 * HSL], BF16, kind="ExternalInput").ap()
    wvT = nc.dram_tensor("wvT", [128, 8 * HSL], BF16, kind="ExternalInput").ap()
    woT = nc.dram_tensor("woT", [128, 4 * HID], BF16, kind="ExternalInput").ap()
    bq = nc.dram_tensor("bq", [HSL], F32, kind="ExternalInput").ap()
    bk = nc.dram_tensor("bk", [HSL], F32, kind="ExternalInput").ap()
    bv_rep = nc.dram_tensor("bv_rep", [128, HSL], F32, kind="ExternalInput").ap()
    dmask = nc.dram_tensor("dmask", [128, 128], BF16, kind="ExternalInput").ap()
    out = nc.dram_tensor("out_p", [S, HID], F32, kind="ExternalOutput").ap()

    xT_r = xT.rearrange("p (c ic t) -> p c ic t", c=NTC, ic=8)  # [128, 4, 8, TC]
    wqT_r = wqT.rearrange("p (ic o) -> p ic o", ic=8)     # [128, 8, HSL]
    wkT_r = wkT.rearrange("p (ic o) -> p ic o", ic=8)
    wvT_r = wvT.rearrange("p (ic o) -> p ic o", ic=8)
    woT_r = woT.rearrange("p (hb o) -> p hb o", hb=4)     # [128, 4, HID]
    bq_r = bq.rearrange("(d p) -> p d", p=128)            # [128, 4]
    bk_r = bk.rearrange("(d p) -> p d", p=128)
    out_r = out.rearrange("(tb p) o -> p tb o", p=128)    # [128, 16, HID]

    with tile.TileContext(nc) as tc:
        with (
            tc.tile_pool(name="per", bufs=1) as per,
            tc.tile_pool(name="xt", bufs=2) as xt_pool,
            tc.tile_pool(name="pt", bufs=4) as pt_pool,
            tc.tile_pool(name="sm", bufs=3) as sm_pool,
            tc.tile_pool(name="ot", bufs=3) as ot_pool,
            tc.tile_pool(name="psq", bufs=2, space="PSUM") as psq,
            tc.tile_pool(name="pss", bufs=2, space="PSUM") as pss,
            tc.tile_pool(name="pso", bufs=2, space="PSUM") as pso,
        ):
            qT_c, kT_c, v_c, aT_c = [], [], [], []
            for c in range(NTC):
                qT_c.append(per.tile([128, 4, TC], BF16, tag=f"qT{c}", name=f"qT{c}"))
                kT_c.append(per.tile([128, 4, TC], BF16, tag=f"kT{c}", name=f"kT{c}"))
                v_c.append(
                    per.tile([128, 4, HPC, DK + 1], BF16, tag=f"v{c}", name=f"v{c}")
                )
                aT_c.append(
                    [
                        per.tile(
                            [128, TC], BF16, tag=f"aT{c}_{db}", name=f"aT{c}_{db}"
                        )
                        for db in range(4)
                    ]
                )
                nc.vector.memset(v_c[c][:, :, :, DK], 1.0)

            bq_sb = per.tile([128, 4], F32, tag="bq")
            nc.sync.dma_start(bq_sb[:], bq_r)
            bk_sb = per.tile([128, 4], F32, tag="bk")
            nc.sync.dma_start(bk_sb[:], bk_r)
            bv_sb = per.tile([128, HSL], F32, tag="bv")
            nc.sync.dma_start(bv_sb[:], bv_rep)
            dm_sb = per.tile([128, 128], BF16, tag="dm")
            nc.sync.dma_start(dm_sb[:], dmask)
            # first QKV matmul needs wq + x chunk 0: load those two first
            wq_sb = per.tile([128, 8, HSL], BF16, tag="wq")
            nc.sync.dma_start(wq_sb[:], wqT_r)
            xt0 = xt_pool.tile([128, 8, TC], BF16, tag="xt", name="xt0")
            nc.sync.dma_start(xt0[:], xT_r[:, 0])
            wk_sb = per.tile([128, 8, HSL], BF16, tag="wk")
            nc.sync.dma_start(wk_sb[:], wkT_r)
            wv_sb = per.tile([128, 8, HSL], BF16, tag="wv")
            nc.sync.dma_start(wv_sb[:], wvT_r)
            wo_sb = per.tile([128, 4, HID], BF16, tag="wo")
            nc.sync.dma_start(wo_sb[:], woT_r)

            def qkv_fillers(c, xt=None):
                """12 filler closures: Q, K (4 dblk each) and V (4 tb) of
                chunk c, each one PSUM accumulation + eviction."""
                if xt is None:
                    xt = xt_pool.tile([128, 8, TC], BF16, tag="xt", name=f"xt{c}")
                    nc.sync.dma_start(xt[:], xT_r[:, c])
                fs = []
                for w_sb, b_sb, dst in (
                    (wq_sb, bq_sb, qT_c[c]),
                    (wk_sb, bk_sb, kT_c[c]),
                ):
                    for dblk in range(4):
                        def fqk(w_sb=w_sb, b_sb=b_sb, dst=dst, dblk=dblk, xt=xt):
                            ps = psq.tile([128, TC], F32, tag="ps", name="ps")
                            for ic in range(8):
                                nc.tensor.matmul(
                                    ps[:],
                                    w_sb[:, ic, ts(dblk, 128)],
                                    xt[:, ic, :],
                                    start=(ic == 0),
                                    stop=(ic == 7),
                                )
                            nc.vector.tensor_tensor(
                                dst[:, dblk, :],
                                ps[:],
                                b_sb[:, dblk : dblk + 1].to_broadcast((128, TC)),
                                ALU.add,
                            )
                        fs.append(fqk)
                for tbl in range(4):
                    def fv(tbl=tbl, c=c, xt=xt):
                        ps = psq.tile([128, TC], F32, tag="ps", name="ps")
                        for ic in range(8):
                            nc.tensor.matmul(
                                ps[:],
                                xt[:, ic, ts(tbl, 128)],
                                wv_sb[:, ic, :],
                                start=(ic == 0),
                                stop=(ic == 7),
                            )
                        nc.vector.tensor_tensor(
                            v_c[c][:, tbl, :, 0:DK],
                            ps.rearrange("p (h d) -> p h d", d=DK),
                            bv_sb.rearrange("p (h d) -> p h d", d=DK),
                            ALU.add,
                        )
                    fs.append(fv)
                return fs

            def oproj_fillers(c):
                """8 filler closures: the out-projection of chunk c, one
                512-column half-block each."""
                fs = []
                for tbl in range(4):
                    for half in range(2):
                        def fo(tbl=tbl, half=half, c=c):
                            tb = c * 4 + tbl
                            ps = psq.tile([128, 512], F32, tag="ps", name="ps")
                            for hb in range(4):
                                nc.tensor.matmul(
                                    ps[:],
                                    aT_c[c][hb][:, ts(tbl, 128)],
                                    wo_sb[:, hb, ts(half, 512)],
                                    start=(hb == 0),
                                    stop=(hb == 3),
                                )
                            ot = ot_pool.tile(
                                [128, 512], F32, tag="ot", name=f"ot{tb}_{half}"
                            )
                            nc.vector.tensor_copy(ot[:], ps[:])
                            nc.sync.dma_start(out_r[:, tb, ts(half, 512)], ot[:])
                        fs.append(fo)
                return fs

            # prologue: QKV of chunk 0 runs dense (nothing to overlap yet)
            for f in qkv_fillers(0, xt=xt0):
                f()

            # Filler assignment per iteration, sized so every attention
            # stretch stays PE-bound (iteration 3 has the most exp work
            # and no next-chunk QKV, so it gets two out-projections).
            for tci in range(NTC):
                qc = tci
                nkb = 4 * qc + 4
                filler = []
                if tci + 1 < NTC:
                    filler += qkv_fillers(tci + 1)
                if tci == 1:
                    filler += oproj_fillers(0)
                elif tci == 3:
                    filler += oproj_fillers(1) + oproj_fillers(2)
                nf = len(filler)
                n_groups = HPC * ((nkb + GRP - 1) // GRP)
                gi = nemit = 0

                for h in range(HPC):
                    dblk, off = h // 2, (h % 2) * DK
                    qT_h = qT_c[qc][off : off + DK, dblk]
                    ops = pso.tile([DK + 1, TC], F32, tag="ops", name="ops")
                    pend = None

                    def emit_pv(pt_tile, kbs):
                        for j, kb in enumerate(kbs):
                            cs = max(0, kb * 128 - qc * TC)
                            nc.tensor.matmul(
                                ops[:, cs:TC],
                                v_c[kb // 4][:, kb % 4, h, :],
                                pt_tile[:, j, cs:TC],
                                start=(kb == 0),
                                stop=(kb == nkb - 1),
                            )

                    for g0 in range(0, nkb, GRP):
                        kbs = tuple(range(g0, min(g0 + GRP, nkb)))
                        sp = pss.tile([128, GRP, TC], F32, tag="sp", name="sp")
                        pt = pt_pool.tile([128, GRP, TC], BF16, tag="pt", name="pt")
                        for j, kb in enumerate(kbs):
                            cs = max(0, kb * 128 - qc * TC)
                            nc.tensor.matmul(
                                sp[:, j, cs:TC],
                                kT_c[kb // 4][off : off + DK, dblk, ts(kb % 4, 128)],
                                qT_h[:, cs:TC],
                                start=True,
                                stop=True,
                            )
                        if kbs[-1] >= 4 * qc:  # group contains diag blocks
                            for j, kb in enumerate(kbs):
                                cs = max(0, kb * 128 - qc * TC)
                                nc.scalar.activation(
                                    pt[:, j, cs:TC],
                                    sp[:, j, cs:TC],
                                    AF.Exp,
                                    scale=0.125,
                                )
                                if kb >= 4 * qc:
                                    nc.vector.tensor_tensor(
                                        pt[:, j, cs : cs + 128],
                                        pt[:, j, cs : cs + 128],
                                        dm_sb[:],
                                        ALU.mult,
                                    )
                        else:
                            nc.scalar.activation(
                                pt[:, 0 : len(kbs), :],
                                sp[:, 0 : len(kbs), :],
                                AF.Exp,
                                scale=0.125,
                            )
                        if pend is not None:
                            emit_pv(*pend)
                        pend = (pt, kbs)
                        gi += 1
                        while nemit < nf and nemit < gi * nf // n_groups:
                            filler[nemit]()
                            nemit += 1
                    emit_pv(*pend)

                    # normalize: divide by the ones-column denominator row
                    rc = sm_pool.tile([1, TC], F32, tag="rc", name="rc")
                    # custom-DVE ops mishandle partition-offset inputs:
                    # stage the sums row at partition 0
                    lsb = sm_pool.tile([1, TC], F32, tag="lsb", name="lsb")
                    nc.vector.tensor_copy(lsb[:], ops[DK : DK + 1, :])
                    nc.vector.reciprocal_approx_fast(rc[:], lsb[:])
                    bcs = sm_pool.tile([DK, TC], F32, tag="bcs", name="bcs")
                    nc.gpsimd.partition_broadcast(bcs[:], rc[:])
                    # 64-partition DVE ops use banks 0-1, whose output
                    # crossbar routes to either SBUF half (reads follow the
                    # source pattern) — odd heads write rows 64..127 direct.
                    nc.vector.tensor_tensor(
                        aT_c[qc][dblk][off : off + DK, :],
                        ops[0:DK, :],
                        bcs[:],
                        ALU.mult,
                    )
                while nemit < nf:
                    filler[nemit]()
                    nemit += 1

            # tail: last chunk's out-projection
            for f in oproj_fillers(NTC - 1):
                f()
    nc.compile()
    return nc


_NC = None


def _get_nc():
    global _NC
    if _NC is None:
        _NC = build_nc()
    return _NC


def _numpy_reference(x, attn_mask, wq, bq, wk, bk, wv, bv, wo, bo):
    """Fallback for a non-causal mask (never hit with the standard inputs)."""
    Bsz, Seq, D = x.shape
    scale = 1.0 / np.sqrt(DK)

    def proj(w, b):
        y = x @ w.T + b
        return y.reshape(Bsz, Seq, HEADS, DK).transpose(0, 2, 1, 3)

    q, k, v = proj(wq, bq), proj(wk, bk), proj(wv, bv)
    scores = np.einsum("bhqd,bhkd->bhqk", q, k) * scale
    scores = np.where(attn_mask == 0, np.float32(-1e9), scores)
    scores = scores - scores.max(axis=-1, keepdims=True)
    p = np.exp(scores)
    p /= p.sum(axis=-1, keepdims=True)
    o = np.einsum("bhqk,bhkd->bhqd", p, v)
    o = o.transpose(0, 2, 1, 3).reshape(Bsz, Seq, D)
    return o @ wo.T + bo


def kernel(x, attn_mask, wq, bq, wk, bk, wv, bv, wo, bo, **_unused):
    x = np.asarray(x, np.float32)
    attn_mask = np.asarray(attn_mask)
    wq, bq = np.asarray(wq, np.float32), np.asarray(bq, np.float32)
    wk, bk = np.asarray(wk, np.float32), np.asarray(bk, np.float32)
    wv, bv = np.asarray(wv, np.float32), np.asarray(bv, np.float32)
    wo, bo = np.asarray(wo, np.float32), np.asarray(bo, np.float32)

    causal = np.array_equal(
        np.asarray(attn_mask).reshape(S, S) != 0, np.tril(np.ones((S, S), bool))
    )
    if not causal:
        return _numpy_reference(x, attn_mask, wq, bq, wk, bk, wv, bv, wo, bo)

    bf = lambda a: np.ascontiguousarray(a).astype(ml_dtypes.bfloat16)  # noqa: E731

    # pack so device partition p's data is one contiguous DRAM run
    def pack_x(xb):      # [1024, 2048] -> [128, (chunk, ic, tok)]
        return bf(xb.T.reshape(8, 128, NTC, TC).transpose(1, 2, 0, 3).reshape(128, -1))

    def pack_w(wT):      # [1024, 512] -> [128, (ic, out)]
        return bf(wT.reshape(8, 128, HSL).transpose(1, 0, 2).reshape(128, -1))

    def pack_wo(woT):    # [512, 1024] -> [128, (hb, out)]
        return bf(woT.reshape(4, 128, HID).transpose(1, 0, 2).reshape(128, -1))

    tri = np.where(
        np.arange(128)[:, None] <= np.arange(128)[None, :], 1.0, 0.0
    ).astype(ml_dtypes.bfloat16)

    in_maps = []
    for c in range(NCORES):
        b, hg = c // 2, c % 2
        sl = slice(hg * HSL, (hg + 1) * HSL)
        in_maps.append(
            {
                "xT": pack_x(x[b]),
                "wqT": pack_w(wq[sl, :].T),
                "wkT": pack_w(wk[sl, :].T),
                "wvT": pack_w(wv[sl, :].T),
                "woT": pack_wo(wo[:, sl].T),
                "bq": np.ascontiguousarray(bq[sl]),
                "bk": np.ascontiguousarray(bk[sl]),
                "bv_rep": np.tile(bv[sl][None, :], (128, 1)),
                "dmask": tri,
            }
        )

    res = run_bass_kernel_spmd(
        _get_nc(), in_maps, core_ids=list(range(NCORES)), **_RUN_KWARGS
    )
    if _RUN_RESULTS is not None:
        _RUN_RESULTS.append(res)

    out = np.empty((B, S, HID), np.float32)
    for b in range(B):
        out[b] = res.results[2 * b]["out_p"] + res.results[2 * b + 1]["out_p"] + bo
    return out


# test.py can set these to enable tracing / inspect profile results.
_RUN_KWARGS = {}
_RUN_RESULTS = None
